# revision 36
# baseline (speedup 1.0000x reference)
"""Trainium2 Bass kernel for nn_AxialShift: 5x conv1x1(192->192) + 2x GroupNorm(1,C)
+ exact gelu + 3 axial channel-chunk shifts, data-parallel over batch (1 sample/core,
8 cores). Self-contained: hardcodes shapes (B=8, C=192, R=32).

v1.7: H-shift folded into K-split matmuls over per-chunk gelu scratches (no
shift copies at all), W-shift via strided sync-DMA gathers + DVE edge copies,
c1-A and t-A SBUF-resident (c1-A pre-shifted for the D-axis consumer), single
[64,1024] 2-bank B-psum tile (one B evac instr per pair), bias folding, fused
sum/square stats via accum_out, all DMAs on sync, bf16 output."""

import os
import numpy as np
import ml_dtypes
from contextlib import ExitStack

import concourse.bass as bass
import concourse.tile as tile
from concourse import bacc
from concourse import mybir
from concourse.bass_utils import run_bass_kernel_spmd

C = 192
CA = 128
CB = 64
R = 32
N = R * R * R     # 32768 flat spatial, n = d*1024 + h*32 + w
T = 512
T2 = 2 * T        # pair width (one D-plane)
NT = N // T
NP = NT // 2      # 32 pairs == 32 D-planes
HALO = 1024       # D-shift halo for c1-A big tile
EPS = 1e-5

f32 = mybir.dt.float32
bf16 = mybir.dt.bfloat16
AF = mybir.ActivationFunctionType
ALU = mybir.AluOpType
AX = mybir.AxisListType
GELU = (AF.Tanh if os.environ.get("SIM_TANH") else AF.Gelu)


def _build():
    nc = bacc.Bacc("TRN2", target_bir_lowering=False, debug=False, num_devices=8)

    dp = lambda name, shape, dt, kind: nc.dram_tensor(name, shape, dt, kind=kind).ap()
    x_d = dp("x", [C, N], bf16, "ExternalInput")
    w1T_d = dp("w1T", [C, C], bf16, "ExternalInput")
    w22T_d = dp("w22T", [C, C], bf16, "ExternalInput")
    w21T_d = dp("w21T", [C, C], bf16, "ExternalInput")
    w23T_d = dp("w23T", [C, C], bf16, "ExternalInput")
    w3T_d = dp("w3T", [C, C], bf16, "ExternalInput")
    vecs_d = {}
    for nm in ("b1", "b23e", "b3", "n1w", "n1b", "n2w", "n2b"):
        vecs_d[nm] = dp(nm, [C, 1], f32, "ExternalInput")
    out_d = dp("out", [C, N], bf16, "ExternalOutput")
    h1_d = dp("h1buf", [C, N], bf16, "Internal")
    c1b_d = dp("c1bbuf", [CB, N], bf16, "Internal")
    c2_d = dp("c2buf", [C, N], bf16, "Internal")
    tb_d = dp("tbbuf", [CB, N], bf16, "Internal")

    with tile.TileContext(nc) as tc, ExitStack() as ctx:
        wp = ctx.enter_context(tc.tile_pool(name="weights", bufs=1))
        vp = ctx.enter_context(tc.tile_pool(name="vecs", bufs=1))
        sp = ctx.enter_context(tc.tile_pool(name="stats", bufs=1))
        bigp = ctx.enter_context(tc.tile_pool(name="big", bufs=1))
        io = ctx.enter_context(tc.tile_pool(name="io", bufs=4))
        ev = ctx.enter_context(tc.tile_pool(name="evac", bufs=4))
        scr = ctx.enter_context(tc.tile_pool(name="scratch", bufs=2))
        pm = ctx.enter_context(tc.tile_pool(name="psA", bufs=2, space="PSUM"))
        pb = ctx.enter_context(tc.tile_pool(name="psB", bufs=2, space="PSUM"))

        c1A = bigp.tile([CA, HALO + N], bf16, tag="c1A")   # pre-shifted D layout
        tA = bigp.tile([CA, N], bf16, tag="tA")

        def load_w(d):
            a = wp.tile([CA, C], bf16, tag=f"w{d.name}A")
            b = wp.tile([CB, C], bf16, tag=f"w{d.name}B")
            nc.sync.dma_start(a[:], d[0:CA, :])
            nc.sync.dma_start(b[:], d[CA:C, :])
            return a, b

        w1A, w1B = load_w(w1T_d)
        w22A, w22B = load_w(w22T_d)
        w21A, w21B = load_w(w21T_d)
        w23A, w23B = load_w(w23T_d)
        w3A, w3B = load_w(w3T_d)

        vecs = {}
        for nm, d in vecs_d.items():
            a = vp.tile([CA, 1], f32, tag=f"v{nm}A")
            b = vp.tile([CB, 1], f32, tag=f"v{nm}B")
            nc.sync.dma_start(a[:], d[0:CA, :])
            nc.sync.dma_start(b[:], d[CA:C, :])
            vecs[nm] = (a, b)

        ones_a = vp.tile([1, CA], f32, tag="onesA")
        ones_b = vp.tile([1, CB], f32, tag="onesB")
        nc.gpsimd.memset(ones_a[:], 1.0)
        nc.gpsimd.memset(ones_b[:], 1.0)

        # PE warmups: absorb weight-DMA semaphore waits before the hot loops
        for wa, wb in ((w1A, w1B), (w22A, w22B), (w21A, w21B),
                       (w23A, w23B), (w3A, w3B)):
            pwA = pm.tile([CA, T2], f32, tag="psA2", name="pwA")
            nc.tensor.matmul(pwA[:, 0:1], wa[:, 0:CA], wa[:, 0:1],
                             start=True, stop=True)
            nc.tensor.matmul(pwA[:, 1:2], wb[0:CB, 0:CA], wb[0:CB, 0:1],
                             start=True, stop=True)

        # stats accumulators (per-pair cols)
        s1A = sp.tile([CA, NP], f32, tag="s1A")
        q1A = sp.tile([CA, NP], f32, tag="q1A")
        s1B = sp.tile([CB, NP], f32, tag="s1B")
        q1B = sp.tile([CB, NP], f32, tag="q1B")
        s2A = sp.tile([CA, NP], f32, tag="s2A")
        q2A = sp.tile([CA, NP], f32, tag="q2A")
        s2B = sp.tile([CB, NP], f32, tag="s2B")
        q2B = sp.tile([CB, NP], f32, tag="q2B")

        # pair conv: psA2 [CA,1024] (2 banks), psB2 [CB,1024] (2 banks); each
        # matmul targets one bank; stationaries grouped for LDW reuse.
        def conv_pair(wA, wB, rA2, rB2, psA2, psB2, rA_parts=None):
            rA = rA_parts if rA_parts is not None else [
                rA2[:, k * T:(k + 1) * T] for k in range(2)]
            rB = [rB2[:, k * T:(k + 1) * T] for k in range(2)]
            for k in range(2):
                nc.tensor.matmul(psA2[:, k * T:(k + 1) * T], wA[:, 0:CA], rA[k],
                                 start=True, stop=False)
            for k in range(2):
                nc.tensor.matmul(psA2[:, k * T:(k + 1) * T], wB[:, 0:CA], rB[k],
                                 start=False, stop=True)
            for k in range(2):
                nc.tensor.matmul(psB2[:, k * T:(k + 1) * T], wA[:, CA:C], rA[k],
                                 start=True, stop=False)
            for k in range(2):
                nc.tensor.matmul(psB2[:, k * T:(k + 1) * T], wB[:, CA:C], rB[k],
                                 start=False, stop=True)

        def mk_ps():
            psA2 = pm.tile([CA, T2], f32, tag="psA2", name="psA2")
            psB2 = pb.tile([CB, T2], f32, tag="psB2", name="psB2")
            return psA2, psB2

        # ---------- Stage 1: h1 = w1 @ x (biasless), stats of h1+b1 ----------
        for p in range(NP):
            o = p * T2
            xa2 = io.tile([CA, T2], bf16, tag="iA")
            xb2 = io.tile([CB, T2], bf16, tag="iB")
            nc.sync.dma_start(xa2[:], x_d[0:CA, o:o + T2])
            nc.sync.dma_start(xb2[:], x_d[CA:C, o:o + T2])
            psA2, psB2 = mk_ps()
            conv_pair(w1A, w1B, xa2, xb2, psA2, psB2)
            hA2 = ev.tile([CA, T2], bf16, tag="eA")
            hB2 = ev.tile([CB, T2], bf16, tag="eB")
            nc.scalar.activation(hA2[:], psA2[:], AF.Identity,
                                 accum_out=s1A[:, p:p + 1])
            nc.vector.tensor_scalar(hB2[:], psB2[:], 0.0, 0.0, ALU.add, ALU.add,
                                    accum_out=s1B[:, p:p + 1])
            sqA = scr.tile([CA, T2], bf16, tag="sqA")
            sqB = scr.tile([CB, T2], bf16, tag="sqB")
            if p % 2 == 0:
                nc.scalar.activation(sqA[:], hA2[:], AF.Square,
                                     accum_out=q1A[:, p:p + 1])
            else:
                nc.vector.tensor_tensor(sqA[:], hA2[:], hA2[:], ALU.mult)
                nc.vector.tensor_reduce(q1A[:, p:p + 1], sqA[:], AX.X, ALU.add)
            nc.scalar.activation(sqB[:], hB2[:], AF.Square,
                                 accum_out=q1B[:, p:p + 1])
            nc.sync.dma_start(h1_d[0:CA, o:o + T2], hA2[:])
            nc.sync.dma_start(h1_d[CA:C, o:o + T2], hB2[:])

        # ---------- stats finalize -> per-channel scale/bias vectors ----------
        def finalize(sA, qA, sB, qB, nw, nb, bfold, tag):
            csA = sp.tile([CA, 1], f32, tag=f"csA{tag}")
            cqA = sp.tile([CA, 1], f32, tag=f"cqA{tag}")
            csB = sp.tile([CB, 1], f32, tag=f"csB{tag}")
            cqB = sp.tile([CB, 1], f32, tag=f"cqB{tag}")
            nc.vector.tensor_reduce(csA[:], sA[:], AX.X, ALU.add)
            nc.vector.tensor_reduce(cqA[:], qA[:], AX.X, ALU.add)
            nc.vector.tensor_reduce(csB[:], sB[:], AX.X, ALU.add)
            nc.vector.tensor_reduce(cqB[:], qB[:], AX.X, ALU.add)
            if bfold is not None:
                # stats were computed on biasless h; correct to h+b:
                # s' = s + N*b ; q' = q + 2*b*s + N*b^2
                for cs, cq, bv, P in ((csA, cqA, bfold[0], CA),
                                      (csB, cqB, bfold[1], CB)):
                    tmp = sp.tile([P, 1], f32, tag=f"bf{tag}{P}")
                    nc.vector.tensor_tensor(tmp[:], bv[:], cs[:], ALU.mult)
                    nc.vector.tensor_scalar_mul(tmp[:], tmp[:], 2.0)
                    nc.vector.tensor_tensor(cq[:], cq[:], tmp[:], ALU.add)
                    nc.vector.tensor_tensor(tmp[:], bv[:], bv[:], ALU.mult)
                    nc.vector.tensor_scalar_mul(tmp[:], tmp[:], float(N))
                    nc.vector.tensor_tensor(cq[:], cq[:], tmp[:], ALU.add)
                    nc.vector.tensor_scalar_mul(tmp[:], bv[:], float(N))
                    nc.vector.tensor_tensor(cs[:], cs[:], tmp[:], ALU.add)
            # cross-partition totals via gpsimd partition-axis reduces
            stA = sp.tile([1, 1], f32, tag=f"stA{tag}")
            stB = sp.tile([1, 1], f32, tag=f"stB{tag}")
            qtA = sp.tile([1, 1], f32, tag=f"qtA{tag}")
            qtB = sp.tile([1, 1], f32, tag=f"qtB{tag}")
            nc.gpsimd.tensor_reduce(stA[:], csA[:], AX.C, ALU.add)
            nc.gpsimd.tensor_reduce(stB[:], csB[:], AX.C, ALU.add)
            nc.gpsimd.tensor_reduce(qtA[:], cqA[:], AX.C, ALU.add)
            nc.gpsimd.tensor_reduce(qtB[:], cqB[:], AX.C, ALU.add)
            stot = sp.tile([1, 1], f32, tag=f"stot{tag}")
            qtot = sp.tile([1, 1], f32, tag=f"qtot{tag}")
            nc.vector.tensor_tensor(stot[:], stA[:], stB[:], ALU.add)
            nc.vector.tensor_tensor(qtot[:], qtA[:], qtB[:], ALU.add)
            inv = 1.0 / float(C * N)
            mu = sp.tile([1, 1], f32, tag=f"mu{tag}")
            ex2 = sp.tile([1, 1], f32, tag=f"ex2{tag}")
            nc.vector.tensor_scalar_mul(mu[:], stot[:], inv)
            nc.vector.tensor_scalar_mul(ex2[:], qtot[:], inv)
            var = sp.tile([1, 1], f32, tag=f"var{tag}")
            nc.vector.tensor_tensor(var[:], mu[:], mu[:], ALU.mult)
            nc.vector.tensor_tensor(var[:], ex2[:], var[:], ALU.subtract)
            nc.vector.tensor_scalar_add(var[:], var[:], EPS)
            rsq = sp.tile([1, 1], f32, tag=f"rsq{tag}")
            nc.vector.reciprocal(rsq[:], var[:])
            rs = sp.tile([1, 1], f32, tag=f"rs{tag}")
            nc.scalar.activation(rs[:], rsq[:], AF.Sqrt)
            nmu = sp.tile([1, 1], f32, tag=f"nmu{tag}")
            nc.vector.tensor_scalar_mul(nmu[:], mu[:], -1.0)
            bc = {}
            for val, vn in ((rs, "rs"), (nmu, "nmu")):
                pA = pm.tile([CA, T2], f32, tag="psA2", name="pA")
                pB = pb.tile([CB, T2], f32, tag="psB2", name="pB")
                nc.tensor.matmul(pA[:, 0:1], ones_a[:], val[:], start=True, stop=True)
                nc.tensor.matmul(pB[:, 0:1], ones_b[:], val[:], start=True, stop=True)
                tA_ = sp.tile([CA, 1], f32, tag=f"bc{vn}A{tag}")
                tB_ = sp.tile([CB, 1], f32, tag=f"bc{vn}B{tag}")
                nc.vector.tensor_copy(tA_[:], pA[:, 0:1])
                nc.vector.tensor_copy(tB_[:], pB[:, 0:1])
                bc[vn] = (tA_, tB_)
            outs = []
            for half in (0, 1):
                P = CA if half == 0 else CB
                sc = sp.tile([P, 1], f32, tag=f"scale{tag}{half}")
                bi = sp.tile([P, 1], f32, tag=f"bias{tag}{half}")
                nc.vector.tensor_tensor(sc[:], bc["rs"][half][:], nw[half][:], ALU.mult)
                if bfold is not None:
                    nc.vector.tensor_tensor(bi[:], bfold[half][:], bc["nmu"][half][:],
                                            ALU.add)
                    nc.vector.tensor_tensor(bi[:], bi[:], sc[:], ALU.mult)
                else:
                    nc.vector.tensor_tensor(bi[:], bc["nmu"][half][:], sc[:], ALU.mult)
                nc.vector.tensor_tensor(bi[:], bi[:], nb[half][:], ALU.add)
                outs += [sc, bi]
            return outs

        sc1A, bi1A, sc1B, bi1B = finalize(s1A, q1A, s1B, q1B,
                                          vecs["n1w"], vecs["n1b"], vecs["b1"], "1")

        # ------- Stage 3: c1 = w22 @ shiftH(gelu(norm1(h1+b1))) -------
        # gelu applied per channel-chunk into [64,1024] scratches; the H-shift
        # is absorbed by K-split matmuls over shifted rhs slices (within-plane,
        # contiguous). c1-A written PRE-SHIFTED for the D-axis consumer.
        # per-chunk norm scale/bias slices (chunk1 = A rows 64..128)
        for p in range(NP):
            o = p * T2
            gA2 = io.tile([CA, T2], bf16, tag="iA")
            gB2 = io.tile([CB, T2], bf16, tag="iB")
            # H-shift gathered by contiguous DMAs (within-plane):
            # chunk0 reads h+1 (reflect h31->h30), chunk2 reads h-1 (h0->h1)
            nc.sync.dma_start(gA2[0:CB, 0:T2 - 32], h1_d[0:CB, o + 32:o + T2])
            nc.sync.dma_start(gA2[0:CB, T2 - 32:T2],
                              h1_d[0:CB, o + T2 - 64:o + T2 - 32])
            nc.sync.dma_start(gA2[CB:CA, :], h1_d[CB:CA, o:o + T2])
            nc.sync.dma_start(gB2[:, 32:T2], h1_d[CA:C, o:o + T2 - 32])
            nc.sync.dma_start(gB2[:, 0:32], h1_d[CA:C, o + 32:o + 64])
            aA2 = io.tile([CA, T2], bf16, tag="aA")
            aB2 = io.tile([CB, T2], bf16, tag="aB")
            nc.scalar.activation(aA2[:], gA2[:], GELU, scale=sc1A[:], bias=bi1A[:])
            nc.scalar.activation(aB2[:], gB2[:], GELU, scale=sc1B[:], bias=bi1B[:])
            psA2, psB2 = mk_ps()
            conv_pair(w22A, w22B, aA2, aB2, psA2, psB2)
            # pre-shifted evac: chunk0 at col base o, chunk1 at HALO+o
            nc.vector.tensor_copy(c1A[0:CB, o:o + T2], psA2[0:CB, :])
            nc.vector.tensor_copy(c1A[CB:CA, HALO + o:HALO + o + T2],
                                  psA2[CB:CA, :])
            cB2 = ev.tile([CB, T2], bf16, tag="eB")
            nc.scalar.activation(cB2[:], psB2[:], AF.Identity)
            nc.sync.dma_start(c1b_d[:, o:o + T2], cB2[:])
        # reflect fixup for chunk0 at plane 31: reader wants plane 30, whose
        # chunk0 store base is col 30*T2 in the pre-shifted layout
        nc.gpsimd.tensor_copy(c1A[0:CB, HALO + 31 * T2:HALO + 32 * T2],
                              c1A[0:CB, 30 * T2:31 * T2])

        # ---------- Stage 4: c2 = w21 @ shiftD(c1) ----------
        for p in range(NP):
            o = p * T2
            op = o - (1024 if p > 0 else -1024)
            gB2 = io.tile([CB, T2], bf16, tag="iB")
            nc.sync.dma_start(gB2[:], c1b_d[:, op:op + T2])
            psA2, psB2 = mk_ps()
            rA = [c1A[:, HALO + o + k * T:HALO + o + (k + 1) * T] for k in range(2)]
            conv_pair(w21A, w21B, None, gB2, psA2, psB2, rA_parts=rA)
            cA2 = ev.tile([CA, T2], bf16, tag="eA")
            cB2 = ev.tile([CB, T2], bf16, tag="eB")
            nc.vector.tensor_scalar(cA2[:], psA2[:], 0.0, None, ALU.add)
            nc.vector.tensor_scalar(cB2[:], psB2[:], 0.0, None, ALU.add)
            nc.sync.dma_start(c2_d[0:CA, o:o + T2], cA2[:])
            nc.sync.dma_start(c2_d[CA:C, o:o + T2], cB2[:])

        # ---- Stage 5: t = gelu(w23 @ shiftW(c2) + b23e), stats of t ----
        # W-shift via strided DMA gathers (sync) + DVE in-tile edge copies.
        for p in range(NP):
            o = p * T2
            gA2 = io.tile([CA, T2], bf16, tag="iA")
            gB2 = io.tile([CB, T2], bf16, tag="iB")
            nc.sync.dma_start(gA2[CB:CA, :], c2_d[CB:CA, o:o + T2])
            c2v0 = c2_d[0:CB, o:o + T2].rearrange("c (r w) -> c r w", w=32)
            c2v2 = c2_d[CA:C, o:o + T2].rearrange("c (r w) -> c r w", w=32)
            gAv = gA2[0:CB, :].rearrange("c (r w) -> c r w", w=32)
            gBv = gB2[:].rearrange("c (r w) -> c r w", w=32)
            nc.scalar.dma_start(gAv[:, :, 0:31], c2v0[:, :, 1:32])
            nc.vector.tensor_copy(gAv[:, :, 31:32], gAv[:, :, 29:30])
            nc.scalar.dma_start(gBv[:, :, 1:32], c2v2[:, :, 0:31])
            nc.vector.tensor_copy(gBv[:, :, 0:1], gBv[:, :, 2:3])
            psA2, psB2 = mk_ps()
            conv_pair(w23A, w23B, gA2, gB2, psA2, psB2)
            tB2 = ev.tile([CB, T2], bf16, tag="eB")
            nc.scalar.activation(tA[:, o:o + T2], psA2[:], GELU,
                                 bias=vecs["b23e"][0][:],
                                 accum_out=s2A[:, p:p + 1])
            nc.scalar.activation(tB2[:], psB2[:], GELU,
                                 bias=vecs["b23e"][1][:],
                                 accum_out=s2B[:, p:p + 1])
            sqA = scr.tile([CA, T2], bf16, tag="sqA")
            sqB = scr.tile([CB, T2], bf16, tag="sqB")
            nc.scalar.activation(sqA[:], tA[:, o:o + T2], AF.Square,
                                 accum_out=q2A[:, p:p + 1])
            nc.vector.tensor_tensor(sqB[:], tB2[:], tB2[:], ALU.mult)
            nc.vector.tensor_reduce(q2B[:, p:p + 1], sqB[:], AX.X, ALU.add)
            nc.scalar.dma_start(tb_d[:, o:o + T2], tB2[:])

        # ---------- stats2 finalize; fold norm2 into w3 ----------
        sc2A, bi2A, sc2B, bi2B = finalize(s2A, q2A, s2B, q2B,
                                          vecs["n2w"], vecs["n2b"], None, "2")
        w3sA = wp.tile([CA, C], bf16, tag="w3sA")
        w3sB = wp.tile([CB, C], bf16, tag="w3sB")
        nc.vector.tensor_scalar_mul(w3sA[:], w3A[:], sc2A[:])
        nc.vector.tensor_scalar_mul(w3sB[:], w3B[:], sc2B[:])
        b2Ab = sp.tile([CA, 1], bf16, tag="b2Ab")
        b2Bb = sp.tile([CB, 1], bf16, tag="b2Bb")
        nc.vector.tensor_copy(b2Ab[:], bi2A[:])
        nc.vector.tensor_copy(b2Bb[:], bi2B[:])
        pyA = pm.tile([CA, T2], f32, tag="psA2", name="pyA")
        pyB = pb.tile([CB, T2], f32, tag="psB2", name="pyB")
        nc.tensor.matmul(pyA[:, 0:1], w3A[:, 0:CA], b2Ab[:], start=True, stop=False)
        nc.tensor.matmul(pyA[:, 0:1], w3B[:, 0:CA], b2Bb[:], start=False, stop=True)
        nc.tensor.matmul(pyB[:, 0:1], w3A[:, CA:C], b2Ab[:], start=True, stop=False)
        nc.tensor.matmul(pyB[:, 0:1], w3B[:, CA:C], b2Bb[:], start=False, stop=True)
        ybA = sp.tile([CA, 1], f32, tag="ybA")
        ybB = sp.tile([CB, 1], f32, tag="ybB")
        nc.vector.tensor_tensor(ybA[:], pyA[:, 0:1], vecs["b3"][0][:], ALU.add)
        nc.vector.tensor_tensor(ybB[:], pyB[:, 0:1], vecs["b3"][1][:], ALU.add)

        # ---------- Stage 7: out = w3s @ t + yb ----------
        for p in range(NP):
            o = p * T2
            tB2 = io.tile([CB, T2], bf16, tag="iB")
            nc.sync.dma_start(tB2[:], tb_d[:, o:o + T2])
            psA2, psB2 = mk_ps()
            rA = [tA[:, o + k * T:o + (k + 1) * T] for k in range(2)]
            conv_pair(w3sA, w3sB, None, tB2, psA2, psB2, rA_parts=rA)
            oA2 = ev.tile([CA, T2], bf16, tag="eA")
            oB2 = ev.tile([CB, T2], bf16, tag="eB")
            nc.scalar.activation(oA2[:], psA2[:], AF.Identity, bias=ybA[:])
            nc.vector.tensor_scalar_add(oB2[:], psB2[:], ybB[:])
            nc.sync.dma_start(out_d[0:CA, o:o + T2], oA2[:])
            nc.sync.dma_start(out_d[CA:C, o:o + T2], oB2[:])

    nc.finalize()
    return nc


def kernel(x, w1, b1, n1w, n1b, w21, b21, w22, b22, w23, b23, n2w, n2b, w3, b3):
    bf = ml_dtypes.bfloat16
    nc = _build()
    col = lambda v: np.ascontiguousarray(np.asarray(v, np.float32).reshape(C, 1))
    # fold conv-chain biases: b23e = b23 + w23 @ (b21 + w21 @ b22)
    b23e = (np.asarray(b23, np.float64)
            + np.asarray(w23, np.float64) @ (np.asarray(b21, np.float64)
                                             + np.asarray(w21, np.float64)
                                             @ np.asarray(b22, np.float64)))
    common = {
        "w1T": np.ascontiguousarray(np.asarray(w1, np.float32).T.astype(bf)),
        "w22T": np.ascontiguousarray(np.asarray(w22, np.float32).T.astype(bf)),
        "w21T": np.ascontiguousarray(np.asarray(w21, np.float32).T.astype(bf)),
        "w23T": np.ascontiguousarray(np.asarray(w23, np.float32).T.astype(bf)),
        "w3T": np.ascontiguousarray(np.asarray(w3, np.float32).T.astype(bf)),
        "b1": col(b1), "b23e": col(b23e.astype(np.float32)), "b3": col(b3),
        "n1w": col(n1w), "n1b": col(n1b), "n2w": col(n2w), "n2b": col(n2b),
    }
    xs = np.asarray(x, np.float32).astype(bf)
    in_maps = [dict(common, x=np.ascontiguousarray(xs[i].reshape(C, N)))
               for i in range(8)]
    trace = bool(os.environ.get("KPROF"))
    ncores = int(os.environ.get("NCORES", "8"))
    res = run_bass_kernel_spmd(nc, in_maps[:ncores], core_ids=list(range(ncores)),
                               trace=trace)
    if trace:
        print("HW exec time:", res.exec_time_ns, "ns")
        print("profile trace_dir:", getattr(res, "profile_json", None))
    outs = [np.asarray(res.results[i]["out"], np.float32).reshape(C, R, R, R)
            for i in range(len(res.results))]
    while len(outs) < 8:
        outs.append(outs[0])
    return np.stack(outs)


# revision 37
# speedup vs baseline: 1.1935x; 1.1935x over previous
"""Trainium2 Bass kernel for nn_AxialShift: 5x conv1x1(192->192) + 2x GroupNorm(1,C)
+ exact gelu + 3 axial channel-chunk shifts, data-parallel over batch (1 sample/core,
8 cores). Self-contained: hardcodes shapes (B=8, C=192, R=32).

v1.7: H-shift folded into K-split matmuls over per-chunk gelu scratches (no
shift copies at all), W-shift via strided sync-DMA gathers + DVE edge copies,
c1-A and t-A SBUF-resident (c1-A pre-shifted for the D-axis consumer), single
[64,1024] 2-bank B-psum tile (one B evac instr per pair), bias folding, fused
sum/square stats via accum_out, all DMAs on sync, bf16 output."""

import os
import numpy as np
import ml_dtypes
from contextlib import ExitStack

import concourse.bass as bass
import concourse.tile as tile
from concourse import bacc
from concourse import mybir
from concourse.bass_utils import run_bass_kernel_spmd

C = 192
CA = 128
CB = 64
R = 32
N = R * R * R     # 32768 flat spatial, n = d*1024 + h*32 + w
T = 512
T2 = 2 * T        # pair width (one D-plane)
NT = N // T
NP = NT // 2      # 32 pairs == 32 D-planes
HALO = 1024       # D-shift halo for c1-A big tile
EPS = 1e-5

f32 = mybir.dt.float32
bf16 = mybir.dt.bfloat16
AF = mybir.ActivationFunctionType
ALU = mybir.AluOpType
AX = mybir.AxisListType
GELU = (AF.Tanh if os.environ.get("SIM_TANH") else AF.Gelu)


def _build():
    nc = bacc.Bacc("TRN2", target_bir_lowering=False, debug=False, num_devices=8)

    dp = lambda name, shape, dt, kind: nc.dram_tensor(name, shape, dt, kind=kind).ap()
    x_d = dp("x", [C, N], bf16, "ExternalInput")
    w1T_d = dp("w1T", [C, C], bf16, "ExternalInput")
    w22T_d = dp("w22T", [C, C], bf16, "ExternalInput")
    w21T_d = dp("w21T", [C, C], bf16, "ExternalInput")
    w23T_d = dp("w23T", [C, C], bf16, "ExternalInput")
    w3T_d = dp("w3T", [C, C], bf16, "ExternalInput")
    vecs_d = {}
    for nm in ("b1", "b23e", "b3", "n1w", "n1b", "n2w", "n2b"):
        vecs_d[nm] = dp(nm, [C, 1], f32, "ExternalInput")
    out_d = dp("out", [C, N], bf16, "ExternalOutput")
    h1_d = dp("h1buf", [C, N], bf16, "Internal")
    c1b_d = dp("c1bbuf", [CB, N], bf16, "Internal")
    c2_d = dp("c2buf", [C, N], bf16, "Internal")
    tb_d = dp("tbbuf", [CB, N], bf16, "Internal")

    with tile.TileContext(nc) as tc, ExitStack() as ctx:
        wp = ctx.enter_context(tc.tile_pool(name="weights", bufs=1))
        vp = ctx.enter_context(tc.tile_pool(name="vecs", bufs=1))
        sp = ctx.enter_context(tc.tile_pool(name="stats", bufs=1))
        bigp = ctx.enter_context(tc.tile_pool(name="big", bufs=1))
        io = ctx.enter_context(tc.tile_pool(name="io", bufs=4))
        ev = ctx.enter_context(tc.tile_pool(name="evac", bufs=4))
        scr = ctx.enter_context(tc.tile_pool(name="scratch", bufs=2))
        pm = ctx.enter_context(tc.tile_pool(name="psA", bufs=2, space="PSUM"))
        pb = ctx.enter_context(tc.tile_pool(name="psB", bufs=2, space="PSUM"))

        c1A = bigp.tile([CA, HALO + N], bf16, tag="c1A")   # pre-shifted D layout
        tA = bigp.tile([CA, N], bf16, tag="tA")

        def load_w(d):
            a = wp.tile([CA, C], bf16, tag=f"w{d.name}A")
            b = wp.tile([CB, C], bf16, tag=f"w{d.name}B")
            nc.sync.dma_start(a[:], d[0:CA, :])
            nc.sync.dma_start(b[:], d[CA:C, :])
            return a, b

        w1A, w1B = load_w(w1T_d)
        w22A, w22B = load_w(w22T_d)
        w21A, w21B = load_w(w21T_d)
        w23A, w23B = load_w(w23T_d)
        w3A, w3B = load_w(w3T_d)

        vecs = {}
        for nm, d in vecs_d.items():
            a = vp.tile([CA, 1], f32, tag=f"v{nm}A")
            b = vp.tile([CB, 1], f32, tag=f"v{nm}B")
            nc.sync.dma_start(a[:], d[0:CA, :])
            nc.sync.dma_start(b[:], d[CA:C, :])
            vecs[nm] = (a, b)

        ones_a = vp.tile([1, CA], f32, tag="onesA")
        ones_b = vp.tile([1, CB], f32, tag="onesB")
        nc.gpsimd.memset(ones_a[:], 1.0)
        nc.gpsimd.memset(ones_b[:], 1.0)

        # PE warmups: absorb weight-DMA semaphore waits before the hot loops
        for wa, wb in ((w1A, w1B), (w22A, w22B), (w21A, w21B),
                       (w23A, w23B), (w3A, w3B)):
            pwA = pm.tile([CA, T2], f32, tag="psA2", name="pwA")
            nc.tensor.matmul(pwA[:, 0:1], wa[:, 0:CA], wa[:, 0:1],
                             start=True, stop=True)
            nc.tensor.matmul(pwA[:, 1:2], wb[0:CB, 0:CA], wb[0:CB, 0:1],
                             start=True, stop=True)

        # stats accumulators (per-pair cols)
        s1A = sp.tile([CA, NP], f32, tag="s1A")
        q1A = sp.tile([CA, NP], f32, tag="q1A")
        s1B = sp.tile([CB, NP], f32, tag="s1B")
        q1B = sp.tile([CB, NP], f32, tag="q1B")
        s2A = sp.tile([CA, NP], f32, tag="s2A")
        q2A = sp.tile([CA, NP], f32, tag="q2A")
        s2B = sp.tile([CB, NP], f32, tag="s2B")
        q2B = sp.tile([CB, NP], f32, tag="q2B")

        # pair conv: psA2 [CA,1024] (2 banks), psB2 [CB,1024] (2 banks); each
        # matmul targets one bank; stationaries grouped for LDW reuse.
        def conv_pair(wA, wB, rA2, rB2, psA2, psB2, rA_parts=None):
            rA = rA_parts if rA_parts is not None else [
                rA2[:, k * T:(k + 1) * T] for k in range(2)]
            rB = [rB2[:, k * T:(k + 1) * T] for k in range(2)]
            for k in range(2):
                nc.tensor.matmul(psA2[:, k * T:(k + 1) * T], wA[:, 0:CA], rA[k],
                                 start=True, stop=False)
            for k in range(2):
                nc.tensor.matmul(psA2[:, k * T:(k + 1) * T], wB[:, 0:CA], rB[k],
                                 start=False, stop=True)
            for k in range(2):
                nc.tensor.matmul(psB2[:, k * T:(k + 1) * T], wA[:, CA:C], rA[k],
                                 start=True, stop=False)
            for k in range(2):
                nc.tensor.matmul(psB2[:, k * T:(k + 1) * T], wB[:, CA:C], rB[k],
                                 start=False, stop=True)

        def mk_ps():
            psA2 = pm.tile([CA, T2], f32, tag="psA2", name="psA2")
            psB2 = pb.tile([CB, T2], f32, tag="psB2", name="psB2")
            return psA2, psB2

        # ---------- Stage 1: h1 = w1 @ x (biasless), stats of h1+b1 ----------
        for p in range(NP):
            o = p * T2
            xa2 = io.tile([CA, T2], bf16, tag="iA")
            xb2 = io.tile([CB, T2], bf16, tag="iB")
            nc.sync.dma_start(xa2[:], x_d[0:CA, o:o + T2])
            nc.sync.dma_start(xb2[:], x_d[CA:C, o:o + T2])
            psA2, psB2 = mk_ps()
            conv_pair(w1A, w1B, xa2, xb2, psA2, psB2)
            hA2 = ev.tile([CA, T2], bf16, tag="eA")
            hB2 = ev.tile([CB, T2], bf16, tag="eB")
            nc.scalar.activation(hA2[:], psA2[:], AF.Identity,
                                 accum_out=s1A[:, p:p + 1])
            nc.vector.tensor_scalar(hB2[:], psB2[:], 0.0, 0.0, ALU.add, ALU.add,
                                    accum_out=s1B[:, p:p + 1])
            sqA = scr.tile([CA, T2], bf16, tag="sqA")
            sqB = scr.tile([CB, T2], bf16, tag="sqB")
            if p % 2 == 0:
                nc.scalar.activation(sqA[:], hA2[:], AF.Square,
                                     accum_out=q1A[:, p:p + 1])
            else:
                nc.vector.tensor_tensor(sqA[:], hA2[:], hA2[:], ALU.mult)
                nc.vector.tensor_reduce(q1A[:, p:p + 1], sqA[:], AX.X, ALU.add)
            nc.scalar.activation(sqB[:], hB2[:], AF.Square,
                                 accum_out=q1B[:, p:p + 1])
            nc.sync.dma_start(h1_d[0:CA, o:o + T2], hA2[:])
            nc.sync.dma_start(h1_d[CA:C, o:o + T2], hB2[:])

        # ---------- stats finalize -> per-channel scale/bias vectors ----------
        def finalize(sA, qA, sB, qB, nw, nb, bfold, tag):
            csA = sp.tile([CA, 1], f32, tag=f"csA{tag}")
            cqA = sp.tile([CA, 1], f32, tag=f"cqA{tag}")
            csB = sp.tile([CB, 1], f32, tag=f"csB{tag}")
            cqB = sp.tile([CB, 1], f32, tag=f"cqB{tag}")
            nc.vector.tensor_reduce(csA[:], sA[:], AX.X, ALU.add)
            nc.vector.tensor_reduce(cqA[:], qA[:], AX.X, ALU.add)
            nc.vector.tensor_reduce(csB[:], sB[:], AX.X, ALU.add)
            nc.vector.tensor_reduce(cqB[:], qB[:], AX.X, ALU.add)
            if bfold is not None:
                # stats were computed on biasless h; correct to h+b:
                # s' = s + N*b ; q' = q + 2*b*s + N*b^2
                for cs, cq, bv, P in ((csA, cqA, bfold[0], CA),
                                      (csB, cqB, bfold[1], CB)):
                    tmp = sp.tile([P, 1], f32, tag=f"bf{tag}{P}")
                    nc.vector.tensor_tensor(tmp[:], bv[:], cs[:], ALU.mult)
                    nc.vector.tensor_scalar_mul(tmp[:], tmp[:], 2.0)
                    nc.vector.tensor_tensor(cq[:], cq[:], tmp[:], ALU.add)
                    nc.vector.tensor_tensor(tmp[:], bv[:], bv[:], ALU.mult)
                    nc.vector.tensor_scalar_mul(tmp[:], tmp[:], float(N))
                    nc.vector.tensor_tensor(cq[:], cq[:], tmp[:], ALU.add)
                    nc.vector.tensor_scalar_mul(tmp[:], bv[:], float(N))
                    nc.vector.tensor_tensor(cs[:], cs[:], tmp[:], ALU.add)
            # cross-partition totals via gpsimd partition-axis reduces
            stA = sp.tile([1, 1], f32, tag=f"stA{tag}")
            stB = sp.tile([1, 1], f32, tag=f"stB{tag}")
            qtA = sp.tile([1, 1], f32, tag=f"qtA{tag}")
            qtB = sp.tile([1, 1], f32, tag=f"qtB{tag}")
            nc.gpsimd.tensor_reduce(stA[:], csA[:], AX.C, ALU.add)
            nc.gpsimd.tensor_reduce(stB[:], csB[:], AX.C, ALU.add)
            nc.gpsimd.tensor_reduce(qtA[:], cqA[:], AX.C, ALU.add)
            nc.gpsimd.tensor_reduce(qtB[:], cqB[:], AX.C, ALU.add)
            stot = sp.tile([1, 1], f32, tag=f"stot{tag}")
            qtot = sp.tile([1, 1], f32, tag=f"qtot{tag}")
            nc.vector.tensor_tensor(stot[:], stA[:], stB[:], ALU.add)
            nc.vector.tensor_tensor(qtot[:], qtA[:], qtB[:], ALU.add)
            inv = 1.0 / float(C * N)
            mu = sp.tile([1, 1], f32, tag=f"mu{tag}")
            ex2 = sp.tile([1, 1], f32, tag=f"ex2{tag}")
            nc.vector.tensor_scalar_mul(mu[:], stot[:], inv)
            nc.vector.tensor_scalar_mul(ex2[:], qtot[:], inv)
            var = sp.tile([1, 1], f32, tag=f"var{tag}")
            nc.vector.tensor_tensor(var[:], mu[:], mu[:], ALU.mult)
            nc.vector.tensor_tensor(var[:], ex2[:], var[:], ALU.subtract)
            nc.vector.tensor_scalar_add(var[:], var[:], EPS)
            rsq = sp.tile([1, 1], f32, tag=f"rsq{tag}")
            nc.vector.reciprocal(rsq[:], var[:])
            rs = sp.tile([1, 1], f32, tag=f"rs{tag}")
            nc.scalar.activation(rs[:], rsq[:], AF.Sqrt)
            nmu = sp.tile([1, 1], f32, tag=f"nmu{tag}")
            nc.vector.tensor_scalar_mul(nmu[:], mu[:], -1.0)
            bc = {}
            for val, vn in ((rs, "rs"), (nmu, "nmu")):
                pA = pm.tile([CA, T2], f32, tag="psA2", name="pA")
                pB = pb.tile([CB, T2], f32, tag="psB2", name="pB")
                nc.tensor.matmul(pA[:, 0:1], ones_a[:], val[:], start=True, stop=True)
                nc.tensor.matmul(pB[:, 0:1], ones_b[:], val[:], start=True, stop=True)
                tA_ = sp.tile([CA, 1], f32, tag=f"bc{vn}A{tag}")
                tB_ = sp.tile([CB, 1], f32, tag=f"bc{vn}B{tag}")
                nc.vector.tensor_copy(tA_[:], pA[:, 0:1])
                nc.vector.tensor_copy(tB_[:], pB[:, 0:1])
                bc[vn] = (tA_, tB_)
            outs = []
            for half in (0, 1):
                P = CA if half == 0 else CB
                sc = sp.tile([P, 1], f32, tag=f"scale{tag}{half}")
                bi = sp.tile([P, 1], f32, tag=f"bias{tag}{half}")
                nc.vector.tensor_tensor(sc[:], bc["rs"][half][:], nw[half][:], ALU.mult)
                if bfold is not None:
                    nc.vector.tensor_tensor(bi[:], bfold[half][:], bc["nmu"][half][:],
                                            ALU.add)
                    nc.vector.tensor_tensor(bi[:], bi[:], sc[:], ALU.mult)
                else:
                    nc.vector.tensor_tensor(bi[:], bc["nmu"][half][:], sc[:], ALU.mult)
                nc.vector.tensor_tensor(bi[:], bi[:], nb[half][:], ALU.add)
                outs += [sc, bi]
            return outs

        sc1A, bi1A, sc1B, bi1B = finalize(s1A, q1A, s1B, q1B,
                                          vecs["n1w"], vecs["n1b"], vecs["b1"], "1")

        # ------- Stage 3: c1 = w22 @ shiftH(gelu(norm1(h1+b1))) -------
        # gelu applied per channel-chunk into [64,1024] scratches; the H-shift
        # is absorbed by K-split matmuls over shifted rhs slices (within-plane,
        # contiguous). c1-A written PRE-SHIFTED for the D-axis consumer.
        # per-chunk norm scale/bias slices (chunk1 = A rows 64..128)
        for p in range(NP):
            o = p * T2
            gA2 = io.tile([CA, T2], bf16, tag="iA")
            gB2 = io.tile([CB, T2], bf16, tag="iB")
            # H-shift gathered by contiguous DMAs (within-plane):
            # chunk0 reads h+1 (reflect h31->h30), chunk2 reads h-1 (h0->h1)
            nc.sync.dma_start(gA2[0:CB, 0:T2 - 32], h1_d[0:CB, o + 32:o + T2])
            nc.sync.dma_start(gA2[0:CB, T2 - 32:T2],
                              h1_d[0:CB, o + T2 - 64:o + T2 - 32])
            nc.sync.dma_start(gA2[CB:CA, :], h1_d[CB:CA, o:o + T2])
            nc.sync.dma_start(gB2[:, 32:T2], h1_d[CA:C, o:o + T2 - 32])
            nc.sync.dma_start(gB2[:, 0:32], h1_d[CA:C, o + 32:o + 64])
            aA2 = io.tile([CA, T2], bf16, tag="aA")
            aB2 = io.tile([CB, T2], bf16, tag="aB")
            nc.scalar.activation(aA2[:], gA2[:], GELU, scale=sc1A[:], bias=bi1A[:])
            nc.scalar.activation(aB2[:], gB2[:], GELU, scale=sc1B[:], bias=bi1B[:])
            psA2, psB2 = mk_ps()
            conv_pair(w22A, w22B, aA2, aB2, psA2, psB2)
            # pre-shifted evac: chunk0 at col base o, chunk1 at HALO+o
            nc.vector.tensor_copy(c1A[0:CB, o:o + T2], psA2[0:CB, :])
            nc.vector.tensor_copy(c1A[CB:CA, HALO + o:HALO + o + T2],
                                  psA2[CB:CA, :])
            cB2 = ev.tile([CB, T2], bf16, tag="eB")
            nc.scalar.activation(cB2[:], psB2[:], AF.Identity)
            nc.sync.dma_start(c1b_d[:, o:o + T2], cB2[:])
        # reflect fixup for chunk0 at plane 31: reader wants plane 30, whose
        # chunk0 store base is col 30*T2 in the pre-shifted layout
        nc.gpsimd.tensor_copy(c1A[0:CB, HALO + 31 * T2:HALO + 32 * T2],
                              c1A[0:CB, 30 * T2:31 * T2])

        # ---------- Stage 4: c2 = w21 @ shiftD(c1) ----------
        for p in range(NP):
            o = p * T2
            op = o - (1024 if p > 0 else -1024)
            gB2 = io.tile([CB, T2], bf16, tag="iB")
            nc.sync.dma_start(gB2[:], c1b_d[:, op:op + T2])
            psA2, psB2 = mk_ps()
            rA = [c1A[:, HALO + o + k * T:HALO + o + (k + 1) * T] for k in range(2)]
            conv_pair(w21A, w21B, None, gB2, psA2, psB2, rA_parts=rA)
            cA2 = ev.tile([CA, T2], bf16, tag="eA")
            cB2 = ev.tile([CB, T2], bf16, tag="eB")
            nc.vector.tensor_scalar(cA2[:], psA2[:], 0.0, None, ALU.add)
            nc.vector.tensor_scalar(cB2[:], psB2[:], 0.0, None, ALU.add)
            nc.sync.dma_start(c2_d[0:CA, o:o + T2], cA2[:])
            nc.sync.dma_start(c2_d[CA:C, o:o + T2], cB2[:])

        # ---- Stage 5: t = gelu(w23 @ shiftW(c2) + b23e), stats of t ----
        # W-shift via strided DMA gathers (sync) + DVE in-tile edge copies.
        for p in range(NP):
            o = p * T2
            gA2 = io.tile([CA, T2], bf16, tag="iA")
            gB2 = io.tile([CB, T2], bf16, tag="iB")
            nc.sync.dma_start(gA2[CB:CA, :], c2_d[CB:CA, o:o + T2])
            c2v0 = c2_d[0:CB, o:o + T2].rearrange("c (r w) -> c r w", w=32)
            c2v2 = c2_d[CA:C, o:o + T2].rearrange("c (r w) -> c r w", w=32)
            gAv = gA2[0:CB, :].rearrange("c (r w) -> c r w", w=32)
            gBv = gB2[:].rearrange("c (r w) -> c r w", w=32)
            nc.sync.dma_start(gAv[:, :, 0:31], c2v0[:, :, 1:32])
            nc.vector.tensor_copy(gAv[:, :, 31:32], gAv[:, :, 29:30])
            nc.sync.dma_start(gBv[:, :, 1:32], c2v2[:, :, 0:31])
            nc.vector.tensor_copy(gBv[:, :, 0:1], gBv[:, :, 2:3])
            psA2, psB2 = mk_ps()
            conv_pair(w23A, w23B, gA2, gB2, psA2, psB2)
            tB2 = ev.tile([CB, T2], bf16, tag="eB")
            nc.scalar.activation(tA[:, o:o + T2], psA2[:], GELU,
                                 bias=vecs["b23e"][0][:],
                                 accum_out=s2A[:, p:p + 1])
            nc.scalar.activation(tB2[:], psB2[:], GELU,
                                 bias=vecs["b23e"][1][:],
                                 accum_out=s2B[:, p:p + 1])
            sqA = scr.tile([CA, T2], bf16, tag="sqA")
            sqB = scr.tile([CB, T2], bf16, tag="sqB")
            nc.scalar.activation(sqA[:], tA[:, o:o + T2], AF.Square,
                                 accum_out=q2A[:, p:p + 1])
            nc.vector.tensor_tensor(sqB[:], tB2[:], tB2[:], ALU.mult)
            nc.vector.tensor_reduce(q2B[:, p:p + 1], sqB[:], AX.X, ALU.add)
            nc.scalar.dma_start(tb_d[:, o:o + T2], tB2[:])

        # ---------- stats2 finalize; fold norm2 into w3 ----------
        sc2A, bi2A, sc2B, bi2B = finalize(s2A, q2A, s2B, q2B,
                                          vecs["n2w"], vecs["n2b"], None, "2")
        w3sA = wp.tile([CA, C], bf16, tag="w3sA")
        w3sB = wp.tile([CB, C], bf16, tag="w3sB")
        nc.vector.tensor_scalar_mul(w3sA[:], w3A[:], sc2A[:])
        nc.vector.tensor_scalar_mul(w3sB[:], w3B[:], sc2B[:])
        b2Ab = sp.tile([CA, 1], bf16, tag="b2Ab")
        b2Bb = sp.tile([CB, 1], bf16, tag="b2Bb")
        nc.vector.tensor_copy(b2Ab[:], bi2A[:])
        nc.vector.tensor_copy(b2Bb[:], bi2B[:])
        pyA = pm.tile([CA, T2], f32, tag="psA2", name="pyA")
        pyB = pb.tile([CB, T2], f32, tag="psB2", name="pyB")
        nc.tensor.matmul(pyA[:, 0:1], w3A[:, 0:CA], b2Ab[:], start=True, stop=False)
        nc.tensor.matmul(pyA[:, 0:1], w3B[:, 0:CA], b2Bb[:], start=False, stop=True)
        nc.tensor.matmul(pyB[:, 0:1], w3A[:, CA:C], b2Ab[:], start=True, stop=False)
        nc.tensor.matmul(pyB[:, 0:1], w3B[:, CA:C], b2Bb[:], start=False, stop=True)
        ybA = sp.tile([CA, 1], f32, tag="ybA")
        ybB = sp.tile([CB, 1], f32, tag="ybB")
        nc.vector.tensor_tensor(ybA[:], pyA[:, 0:1], vecs["b3"][0][:], ALU.add)
        nc.vector.tensor_tensor(ybB[:], pyB[:, 0:1], vecs["b3"][1][:], ALU.add)

        # ---------- Stage 7: out = w3s @ t + yb ----------
        for p in range(NP):
            o = p * T2
            tB2 = io.tile([CB, T2], bf16, tag="iB")
            nc.sync.dma_start(tB2[:], tb_d[:, o:o + T2])
            psA2, psB2 = mk_ps()
            rA = [tA[:, o + k * T:o + (k + 1) * T] for k in range(2)]
            conv_pair(w3sA, w3sB, None, tB2, psA2, psB2, rA_parts=rA)
            oA2 = ev.tile([CA, T2], bf16, tag="eA")
            oB2 = ev.tile([CB, T2], bf16, tag="eB")
            nc.scalar.activation(oA2[:], psA2[:], AF.Identity, bias=ybA[:])
            nc.vector.tensor_scalar_add(oB2[:], psB2[:], ybB[:])
            nc.sync.dma_start(out_d[0:CA, o:o + T2], oA2[:])
            nc.sync.dma_start(out_d[CA:C, o:o + T2], oB2[:])

    nc.finalize()
    return nc


def kernel(x, w1, b1, n1w, n1b, w21, b21, w22, b22, w23, b23, n2w, n2b, w3, b3):
    bf = ml_dtypes.bfloat16
    nc = _build()
    col = lambda v: np.ascontiguousarray(np.asarray(v, np.float32).reshape(C, 1))
    # fold conv-chain biases: b23e = b23 + w23 @ (b21 + w21 @ b22)
    b23e = (np.asarray(b23, np.float64)
            + np.asarray(w23, np.float64) @ (np.asarray(b21, np.float64)
                                             + np.asarray(w21, np.float64)
                                             @ np.asarray(b22, np.float64)))
    common = {
        "w1T": np.ascontiguousarray(np.asarray(w1, np.float32).T.astype(bf)),
        "w22T": np.ascontiguousarray(np.asarray(w22, np.float32).T.astype(bf)),
        "w21T": np.ascontiguousarray(np.asarray(w21, np.float32).T.astype(bf)),
        "w23T": np.ascontiguousarray(np.asarray(w23, np.float32).T.astype(bf)),
        "w3T": np.ascontiguousarray(np.asarray(w3, np.float32).T.astype(bf)),
        "b1": col(b1), "b23e": col(b23e.astype(np.float32)), "b3": col(b3),
        "n1w": col(n1w), "n1b": col(n1b), "n2w": col(n2w), "n2b": col(n2b),
    }
    xs = np.asarray(x, np.float32).astype(bf)
    in_maps = [dict(common, x=np.ascontiguousarray(xs[i].reshape(C, N)))
               for i in range(8)]
    trace = bool(os.environ.get("KPROF"))
    ncores = int(os.environ.get("NCORES", "8"))
    res = run_bass_kernel_spmd(nc, in_maps[:ncores], core_ids=list(range(ncores)),
                               trace=trace)
    if trace:
        print("HW exec time:", res.exec_time_ns, "ns")
        print("profile trace_dir:", getattr(res, "profile_json", None))
    outs = [np.asarray(res.results[i]["out"], np.float32).reshape(C, R, R, R)
            for i in range(len(res.results))]
    while len(outs) < 8:
        outs.append(outs[0])
    return np.stack(outs)


# revision 38
# speedup vs baseline: 1.2201x; 1.0223x over previous
"""Trainium2 Bass kernel for nn_AxialShift: 5x conv1x1(192->192) + 2x GroupNorm(1,C)
+ exact gelu + 3 axial channel-chunk shifts, data-parallel over batch (1 sample/core,
8 cores). Self-contained: hardcodes shapes (B=8, C=192, R=32).

v1.7: H-shift folded into K-split matmuls over per-chunk gelu scratches (no
shift copies at all), W-shift via strided sync-DMA gathers + DVE edge copies,
c1-A and t-A SBUF-resident (c1-A pre-shifted for the D-axis consumer), single
[64,1024] 2-bank B-psum tile (one B evac instr per pair), bias folding, fused
sum/square stats via accum_out, all DMAs on sync, bf16 output."""

import os
import numpy as np
import ml_dtypes
from contextlib import ExitStack

import concourse.bass as bass
import concourse.tile as tile
from concourse import bacc
from concourse import mybir
from concourse.bass_utils import run_bass_kernel_spmd

C = 192
CA = 128
CB = 64
R = 32
N = R * R * R     # 32768 flat spatial, n = d*1024 + h*32 + w
T = 512
T2 = 2 * T        # pair width (one D-plane)
NT = N // T
NP = NT // 2      # 32 pairs == 32 D-planes
HALO = 1024       # D-shift halo for c1-A big tile
EPS = 1e-5

f32 = mybir.dt.float32
bf16 = mybir.dt.bfloat16
AF = mybir.ActivationFunctionType
ALU = mybir.AluOpType
AX = mybir.AxisListType
GELU = (AF.Tanh if os.environ.get("SIM_TANH") else AF.Gelu)


def _build():
    nc = bacc.Bacc("TRN2", target_bir_lowering=False, debug=False, num_devices=8)

    dp = lambda name, shape, dt, kind: nc.dram_tensor(name, shape, dt, kind=kind).ap()
    x_d = dp("x", [C, N], bf16, "ExternalInput")
    w1T_d = dp("w1T", [C, C], bf16, "ExternalInput")
    w22T_d = dp("w22T", [C, C], bf16, "ExternalInput")
    w21T_d = dp("w21T", [C, C], bf16, "ExternalInput")
    w23T_d = dp("w23T", [C, C], bf16, "ExternalInput")
    w3T_d = dp("w3T", [C, C], bf16, "ExternalInput")
    vecs_d = {}
    for nm in ("b1", "b23e", "b3", "n1w", "n1b", "n2w", "n2b"):
        vecs_d[nm] = dp(nm, [C, 1], f32, "ExternalInput")
    out_d = dp("out", [C, N], bf16, "ExternalOutput")
    h1_d = dp("h1buf", [C, N], bf16, "Internal")
    c1b_d = dp("c1bbuf", [CB, N], bf16, "Internal")
    c2_d = dp("c2buf", [C, N], bf16, "Internal")
    tb_d = dp("tbbuf", [CB, N], bf16, "Internal")

    with tile.TileContext(nc) as tc, ExitStack() as ctx:
        wp = ctx.enter_context(tc.tile_pool(name="weights", bufs=1))
        vp = ctx.enter_context(tc.tile_pool(name="vecs", bufs=1))
        sp = ctx.enter_context(tc.tile_pool(name="stats", bufs=1))
        bigp = ctx.enter_context(tc.tile_pool(name="big", bufs=1))
        io = ctx.enter_context(tc.tile_pool(name="io", bufs=4))
        ev = ctx.enter_context(tc.tile_pool(name="evac", bufs=4))
        scr = ctx.enter_context(tc.tile_pool(name="scratch", bufs=2))
        pm = ctx.enter_context(tc.tile_pool(name="psA", bufs=2, space="PSUM"))
        pb = ctx.enter_context(tc.tile_pool(name="psB", bufs=2, space="PSUM"))

        c1A = bigp.tile([CA, HALO + N], bf16, tag="c1A")   # pre-shifted D layout
        tA = bigp.tile([CA, N], bf16, tag="tA")

        def load_w(d):
            a = wp.tile([CA, C], bf16, tag=f"w{d.name}A")
            b = wp.tile([CB, C], bf16, tag=f"w{d.name}B")
            nc.sync.dma_start(a[:], d[0:CA, :])
            nc.sync.dma_start(b[:], d[CA:C, :])
            return a, b

        w1A, w1B = load_w(w1T_d)
        w22A, w22B = load_w(w22T_d)
        w21A, w21B = load_w(w21T_d)
        w23A, w23B = load_w(w23T_d)
        w3A, w3B = load_w(w3T_d)

        vecs = {}
        for nm, d in vecs_d.items():
            a = vp.tile([CA, 1], f32, tag=f"v{nm}A")
            b = vp.tile([CB, 1], f32, tag=f"v{nm}B")
            nc.sync.dma_start(a[:], d[0:CA, :])
            nc.sync.dma_start(b[:], d[CA:C, :])
            vecs[nm] = (a, b)

        ones_a = vp.tile([1, CA], f32, tag="onesA")
        ones_b = vp.tile([1, CB], f32, tag="onesB")
        nc.gpsimd.memset(ones_a[:], 1.0)
        nc.gpsimd.memset(ones_b[:], 1.0)

        # PE warmups: absorb weight-DMA semaphore waits before the hot loops
        for wa, wb in ((w1A, w1B), (w22A, w22B), (w21A, w21B),
                       (w23A, w23B), (w3A, w3B)):
            pwA = pm.tile([CA, T2], f32, tag="psA2", name="pwA")
            nc.tensor.matmul(pwA[:, 0:1], wa[:, 0:CA], wa[:, 0:1],
                             start=True, stop=True)
            nc.tensor.matmul(pwA[:, 1:2], wb[0:CB, 0:CA], wb[0:CB, 0:1],
                             start=True, stop=True)

        # stats accumulators (per-pair cols)
        s1A = sp.tile([CA, NP], f32, tag="s1A")
        q1A = sp.tile([CA, NP], f32, tag="q1A")
        s1B = sp.tile([CB, NP], f32, tag="s1B")
        q1B = sp.tile([CB, NP], f32, tag="q1B")
        s2A = sp.tile([CA, NP], f32, tag="s2A")
        q2A = sp.tile([CA, NP], f32, tag="q2A")
        s2B = sp.tile([CB, NP], f32, tag="s2B")
        q2B = sp.tile([CB, NP], f32, tag="q2B")

        # pair conv: psA2 [CA,1024] (2 banks), psB2 [CB,1024] (2 banks); each
        # matmul targets one bank; stationaries grouped for LDW reuse.
        def conv_pair(wA, wB, rA2, rB2, psA2, psB2, rA_parts=None):
            rA = rA_parts if rA_parts is not None else [
                rA2[:, k * T:(k + 1) * T] for k in range(2)]
            rB = [rB2[:, k * T:(k + 1) * T] for k in range(2)]
            for k in range(2):
                nc.tensor.matmul(psA2[:, k * T:(k + 1) * T], wA[:, 0:CA], rA[k],
                                 start=True, stop=False)
            for k in range(2):
                nc.tensor.matmul(psA2[:, k * T:(k + 1) * T], wB[:, 0:CA], rB[k],
                                 start=False, stop=True)
            for k in range(2):
                nc.tensor.matmul(psB2[:, k * T:(k + 1) * T], wA[:, CA:C], rA[k],
                                 start=True, stop=False)
            for k in range(2):
                nc.tensor.matmul(psB2[:, k * T:(k + 1) * T], wB[:, CA:C], rB[k],
                                 start=False, stop=True)

        def mk_ps():
            psA2 = pm.tile([CA, T2], f32, tag="psA2", name="psA2")
            psB2 = pb.tile([CB, T2], f32, tag="psB2", name="psB2")
            return psA2, psB2

        # ---------- Stage 1: h1 = w1 @ x (biasless), stats of h1+b1 ----------
        for p in range(NP):
            o = p * T2
            xa2 = io.tile([CA, T2], bf16, tag="iA")
            xb2 = io.tile([CB, T2], bf16, tag="iB")
            nc.sync.dma_start(xa2[:], x_d[0:CA, o:o + T2])
            nc.sync.dma_start(xb2[:], x_d[CA:C, o:o + T2])
            psA2, psB2 = mk_ps()
            conv_pair(w1A, w1B, xa2, xb2, psA2, psB2)
            hA2 = ev.tile([CA, T2], bf16, tag="eA")
            hB2 = ev.tile([CB, T2], bf16, tag="eB")
            nc.scalar.activation(hA2[:], psA2[:], AF.Identity,
                                 accum_out=s1A[:, p:p + 1])
            nc.vector.tensor_scalar(hB2[:], psB2[:], 0.0, 0.0, ALU.add, ALU.add,
                                    accum_out=s1B[:, p:p + 1])
            sqA = scr.tile([CA, T2], bf16, tag="sqA")
            sqB = scr.tile([CB, T2], bf16, tag="sqB")
            if p % 2 == 0:
                nc.scalar.activation(sqA[:], hA2[:], AF.Square,
                                     accum_out=q1A[:, p:p + 1])
            else:
                nc.vector.tensor_tensor(sqA[:], hA2[:], hA2[:], ALU.mult)
                nc.vector.tensor_reduce(q1A[:, p:p + 1], sqA[:], AX.X, ALU.add)
            nc.scalar.activation(sqB[:], hB2[:], AF.Square,
                                 accum_out=q1B[:, p:p + 1])
            nc.sync.dma_start(h1_d[0:CA, o:o + T2], hA2[:])
            nc.sync.dma_start(h1_d[CA:C, o:o + T2], hB2[:])

        # ---------- stats finalize -> per-channel scale/bias vectors ----------
        def finalize(sA, qA, sB, qB, nw, nb, bfold, tag):
            csA = sp.tile([CA, 1], f32, tag=f"csA{tag}")
            cqA = sp.tile([CA, 1], f32, tag=f"cqA{tag}")
            csB = sp.tile([CB, 1], f32, tag=f"csB{tag}")
            cqB = sp.tile([CB, 1], f32, tag=f"cqB{tag}")
            nc.vector.tensor_reduce(csA[:], sA[:], AX.X, ALU.add)
            nc.vector.tensor_reduce(cqA[:], qA[:], AX.X, ALU.add)
            nc.vector.tensor_reduce(csB[:], sB[:], AX.X, ALU.add)
            nc.vector.tensor_reduce(cqB[:], qB[:], AX.X, ALU.add)
            if bfold is not None:
                # stats were computed on biasless h; correct to h+b:
                # s' = s + N*b ; q' = q + 2*b*s + N*b^2
                for cs, cq, bv, P in ((csA, cqA, bfold[0], CA),
                                      (csB, cqB, bfold[1], CB)):
                    tmp = sp.tile([P, 1], f32, tag=f"bf{tag}{P}")
                    nc.vector.tensor_tensor(tmp[:], bv[:], cs[:], ALU.mult)
                    nc.vector.tensor_scalar_mul(tmp[:], tmp[:], 2.0)
                    nc.vector.tensor_tensor(cq[:], cq[:], tmp[:], ALU.add)
                    nc.vector.tensor_tensor(tmp[:], bv[:], bv[:], ALU.mult)
                    nc.vector.tensor_scalar_mul(tmp[:], tmp[:], float(N))
                    nc.vector.tensor_tensor(cq[:], cq[:], tmp[:], ALU.add)
                    nc.vector.tensor_scalar_mul(tmp[:], bv[:], float(N))
                    nc.vector.tensor_tensor(cs[:], cs[:], tmp[:], ALU.add)
            # cross-partition totals via gpsimd partition-axis reduces
            stA = sp.tile([1, 1], f32, tag=f"stA{tag}")
            stB = sp.tile([1, 1], f32, tag=f"stB{tag}")
            qtA = sp.tile([1, 1], f32, tag=f"qtA{tag}")
            qtB = sp.tile([1, 1], f32, tag=f"qtB{tag}")
            nc.gpsimd.tensor_reduce(stA[:], csA[:], AX.C, ALU.add)
            nc.gpsimd.tensor_reduce(stB[:], csB[:], AX.C, ALU.add)
            nc.gpsimd.tensor_reduce(qtA[:], cqA[:], AX.C, ALU.add)
            nc.gpsimd.tensor_reduce(qtB[:], cqB[:], AX.C, ALU.add)
            stot = sp.tile([1, 1], f32, tag=f"stot{tag}")
            qtot = sp.tile([1, 1], f32, tag=f"qtot{tag}")
            nc.vector.tensor_tensor(stot[:], stA[:], stB[:], ALU.add)
            nc.vector.tensor_tensor(qtot[:], qtA[:], qtB[:], ALU.add)
            inv = 1.0 / float(C * N)
            mu = sp.tile([1, 1], f32, tag=f"mu{tag}")
            ex2 = sp.tile([1, 1], f32, tag=f"ex2{tag}")
            nc.vector.tensor_scalar_mul(mu[:], stot[:], inv)
            nc.vector.tensor_scalar_mul(ex2[:], qtot[:], inv)
            var = sp.tile([1, 1], f32, tag=f"var{tag}")
            nc.vector.tensor_tensor(var[:], mu[:], mu[:], ALU.mult)
            nc.vector.tensor_tensor(var[:], ex2[:], var[:], ALU.subtract)
            nc.vector.tensor_scalar_add(var[:], var[:], EPS)
            rsq = sp.tile([1, 1], f32, tag=f"rsq{tag}")
            nc.vector.reciprocal(rsq[:], var[:])
            rs = sp.tile([1, 1], f32, tag=f"rs{tag}")
            nc.scalar.activation(rs[:], rsq[:], AF.Sqrt)
            nmu = sp.tile([1, 1], f32, tag=f"nmu{tag}")
            nc.vector.tensor_scalar_mul(nmu[:], mu[:], -1.0)
            bc = {}
            for val, vn in ((rs, "rs"), (nmu, "nmu")):
                pA = pm.tile([CA, T2], f32, tag="psA2", name="pA")
                pB = pb.tile([CB, T2], f32, tag="psB2", name="pB")
                nc.tensor.matmul(pA[:, 0:1], ones_a[:], val[:], start=True, stop=True)
                nc.tensor.matmul(pB[:, 0:1], ones_b[:], val[:], start=True, stop=True)
                tA_ = sp.tile([CA, 1], f32, tag=f"bc{vn}A{tag}")
                tB_ = sp.tile([CB, 1], f32, tag=f"bc{vn}B{tag}")
                nc.vector.tensor_copy(tA_[:], pA[:, 0:1])
                nc.vector.tensor_copy(tB_[:], pB[:, 0:1])
                bc[vn] = (tA_, tB_)
            outs = []
            for half in (0, 1):
                P = CA if half == 0 else CB
                sc = sp.tile([P, 1], f32, tag=f"scale{tag}{half}")
                bi = sp.tile([P, 1], f32, tag=f"bias{tag}{half}")
                nc.vector.tensor_tensor(sc[:], bc["rs"][half][:], nw[half][:], ALU.mult)
                if bfold is not None:
                    nc.vector.tensor_tensor(bi[:], bfold[half][:], bc["nmu"][half][:],
                                            ALU.add)
                    nc.vector.tensor_tensor(bi[:], bi[:], sc[:], ALU.mult)
                else:
                    nc.vector.tensor_tensor(bi[:], bc["nmu"][half][:], sc[:], ALU.mult)
                nc.vector.tensor_tensor(bi[:], bi[:], nb[half][:], ALU.add)
                outs += [sc, bi]
            return outs

        sc1A, bi1A, sc1B, bi1B = finalize(s1A, q1A, s1B, q1B,
                                          vecs["n1w"], vecs["n1b"], vecs["b1"], "1")

        # ------- Stage 3: c1 = w22 @ shiftH(gelu(norm1(h1+b1))) -------
        # gelu applied per channel-chunk into [64,1024] scratches; the H-shift
        # is absorbed by K-split matmuls over shifted rhs slices (within-plane,
        # contiguous). c1-A written PRE-SHIFTED for the D-axis consumer.
        # per-chunk norm scale/bias slices (chunk1 = A rows 64..128)
        for p in range(NP):
            o = p * T2
            gA2 = io.tile([CA, T2], bf16, tag="iA")
            gB2 = io.tile([CB, T2], bf16, tag="iB")
            # H-shift gathered by contiguous DMAs (within-plane):
            # chunk0 reads h+1 (reflect h31->h30), chunk2 reads h-1 (h0->h1)
            nc.sync.dma_start(gA2[0:CB, 0:T2 - 32], h1_d[0:CB, o + 32:o + T2])
            nc.sync.dma_start(gA2[0:CB, T2 - 32:T2],
                              h1_d[0:CB, o + T2 - 64:o + T2 - 32])
            nc.sync.dma_start(gA2[CB:CA, :], h1_d[CB:CA, o:o + T2])
            nc.sync.dma_start(gB2[:, 32:T2], h1_d[CA:C, o:o + T2 - 32])
            nc.sync.dma_start(gB2[:, 0:32], h1_d[CA:C, o + 32:o + 64])
            aA2 = io.tile([CA, T2], bf16, tag="aA")
            aB2 = io.tile([CB, T2], bf16, tag="aB")
            nc.scalar.activation(aA2[:], gA2[:], GELU, scale=sc1A[:], bias=bi1A[:])
            nc.scalar.activation(aB2[:], gB2[:], GELU, scale=sc1B[:], bias=bi1B[:])
            psA2, psB2 = mk_ps()
            conv_pair(w22A, w22B, aA2, aB2, psA2, psB2)
            # pre-shifted evac: chunk0 at col base o, chunk1 at HALO+o
            nc.vector.tensor_copy(c1A[0:CB, o:o + T2], psA2[0:CB, :])
            nc.vector.tensor_copy(c1A[CB:CA, HALO + o:HALO + o + T2],
                                  psA2[CB:CA, :])
            cB2 = ev.tile([CB, T2], bf16, tag="eB")
            nc.scalar.activation(cB2[:], psB2[:], AF.Identity)
            nc.sync.dma_start(c1b_d[:, o:o + T2], cB2[:])
        # reflect fixup for chunk0 at plane 31: reader wants plane 30, whose
        # chunk0 store base is col 30*T2 in the pre-shifted layout
        nc.gpsimd.tensor_copy(c1A[0:CB, HALO + 31 * T2:HALO + 32 * T2],
                              c1A[0:CB, 30 * T2:31 * T2])

        # ---------- Stage 4: c2 = w21 @ shiftD(c1) ----------
        for p in range(NP):
            o = p * T2
            op = o - (1024 if p > 0 else -1024)
            gB2 = io.tile([CB, T2], bf16, tag="iB")
            nc.sync.dma_start(gB2[:], c1b_d[:, op:op + T2])
            psA2, psB2 = mk_ps()
            rA = [c1A[:, HALO + o + k * T:HALO + o + (k + 1) * T] for k in range(2)]
            conv_pair(w21A, w21B, None, gB2, psA2, psB2, rA_parts=rA)
            cA2 = ev.tile([CA, T2], bf16, tag="eA")
            cB2 = ev.tile([CB, T2], bf16, tag="eB")
            nc.vector.tensor_scalar(cA2[:], psA2[:], 0.0, None, ALU.add)
            nc.vector.tensor_scalar(cB2[:], psB2[:], 0.0, None, ALU.add)
            nc.sync.dma_start(c2_d[0:CA, o:o + T2], cA2[:])
            nc.sync.dma_start(c2_d[CA:C, o:o + T2], cB2[:])

        # ---- Stage 5: t = gelu(w23 @ shiftW(c2) + b23e), stats of t ----
        # W-shift via strided DMA gathers (sync) + DVE in-tile edge copies.
        for p in range(NP):
            o = p * T2
            gA2 = io.tile([CA, T2], bf16, tag="iA")
            gB2 = io.tile([CB, T2], bf16, tag="iB")
            nc.sync.dma_start(gA2[CB:CA, :], c2_d[CB:CA, o:o + T2])
            c2v0 = c2_d[0:CB, o:o + T2].rearrange("c (r w) -> c r w", w=32)
            c2v2 = c2_d[CA:C, o:o + T2].rearrange("c (r w) -> c r w", w=32)
            gAv = gA2[0:CB, :].rearrange("c (r w) -> c r w", w=32)
            gBv = gB2[:].rearrange("c (r w) -> c r w", w=32)
            nc.scalar.dma_start(gAv[:, :, 0:31], c2v0[:, :, 1:32])
            nc.vector.tensor_copy(gAv[:, :, 31:32], gAv[:, :, 29:30])
            nc.sync.dma_start(gBv[:, :, 1:32], c2v2[:, :, 0:31])
            nc.vector.tensor_copy(gBv[:, :, 0:1], gBv[:, :, 2:3])
            psA2, psB2 = mk_ps()
            conv_pair(w23A, w23B, gA2, gB2, psA2, psB2)
            tB2 = ev.tile([CB, T2], bf16, tag="eB")
            nc.scalar.activation(tA[:, o:o + T2], psA2[:], GELU,
                                 bias=vecs["b23e"][0][:],
                                 accum_out=s2A[:, p:p + 1])
            nc.scalar.activation(tB2[:], psB2[:], GELU,
                                 bias=vecs["b23e"][1][:],
                                 accum_out=s2B[:, p:p + 1])
            sqA = scr.tile([CA, T2], bf16, tag="sqA")
            sqB = scr.tile([CB, T2], bf16, tag="sqB")
            nc.scalar.activation(sqA[:], tA[:, o:o + T2], AF.Square,
                                 accum_out=q2A[:, p:p + 1])
            nc.vector.tensor_tensor(sqB[:], tB2[:], tB2[:], ALU.mult)
            nc.vector.tensor_reduce(q2B[:, p:p + 1], sqB[:], AX.X, ALU.add)
            nc.scalar.dma_start(tb_d[:, o:o + T2], tB2[:])

        # ---------- stats2 finalize; fold norm2 into w3 ----------
        sc2A, bi2A, sc2B, bi2B = finalize(s2A, q2A, s2B, q2B,
                                          vecs["n2w"], vecs["n2b"], None, "2")
        w3sA = wp.tile([CA, C], bf16, tag="w3sA")
        w3sB = wp.tile([CB, C], bf16, tag="w3sB")
        nc.vector.tensor_scalar_mul(w3sA[:], w3A[:], sc2A[:])
        nc.vector.tensor_scalar_mul(w3sB[:], w3B[:], sc2B[:])
        b2Ab = sp.tile([CA, 1], bf16, tag="b2Ab")
        b2Bb = sp.tile([CB, 1], bf16, tag="b2Bb")
        nc.vector.tensor_copy(b2Ab[:], bi2A[:])
        nc.vector.tensor_copy(b2Bb[:], bi2B[:])
        pyA = pm.tile([CA, T2], f32, tag="psA2", name="pyA")
        pyB = pb.tile([CB, T2], f32, tag="psB2", name="pyB")
        nc.tensor.matmul(pyA[:, 0:1], w3A[:, 0:CA], b2Ab[:], start=True, stop=False)
        nc.tensor.matmul(pyA[:, 0:1], w3B[:, 0:CA], b2Bb[:], start=False, stop=True)
        nc.tensor.matmul(pyB[:, 0:1], w3A[:, CA:C], b2Ab[:], start=True, stop=False)
        nc.tensor.matmul(pyB[:, 0:1], w3B[:, CA:C], b2Bb[:], start=False, stop=True)
        ybA = sp.tile([CA, 1], f32, tag="ybA")
        ybB = sp.tile([CB, 1], f32, tag="ybB")
        nc.vector.tensor_tensor(ybA[:], pyA[:, 0:1], vecs["b3"][0][:], ALU.add)
        nc.vector.tensor_tensor(ybB[:], pyB[:, 0:1], vecs["b3"][1][:], ALU.add)

        # ---------- Stage 7: out = w3s @ t + yb ----------
        for p in range(NP):
            o = p * T2
            tB2 = io.tile([CB, T2], bf16, tag="iB")
            nc.sync.dma_start(tB2[:], tb_d[:, o:o + T2])
            psA2, psB2 = mk_ps()
            rA = [tA[:, o + k * T:o + (k + 1) * T] for k in range(2)]
            conv_pair(w3sA, w3sB, None, tB2, psA2, psB2, rA_parts=rA)
            oA2 = ev.tile([CA, T2], bf16, tag="eA")
            oB2 = ev.tile([CB, T2], bf16, tag="eB")
            nc.scalar.activation(oA2[:], psA2[:], AF.Identity, bias=ybA[:])
            nc.vector.tensor_scalar_add(oB2[:], psB2[:], ybB[:])
            nc.sync.dma_start(out_d[0:CA, o:o + T2], oA2[:])
            nc.sync.dma_start(out_d[CA:C, o:o + T2], oB2[:])

    nc.finalize()
    return nc


def kernel(x, w1, b1, n1w, n1b, w21, b21, w22, b22, w23, b23, n2w, n2b, w3, b3):
    bf = ml_dtypes.bfloat16
    nc = _build()
    col = lambda v: np.ascontiguousarray(np.asarray(v, np.float32).reshape(C, 1))
    # fold conv-chain biases: b23e = b23 + w23 @ (b21 + w21 @ b22)
    b23e = (np.asarray(b23, np.float64)
            + np.asarray(w23, np.float64) @ (np.asarray(b21, np.float64)
                                             + np.asarray(w21, np.float64)
                                             @ np.asarray(b22, np.float64)))
    common = {
        "w1T": np.ascontiguousarray(np.asarray(w1, np.float32).T.astype(bf)),
        "w22T": np.ascontiguousarray(np.asarray(w22, np.float32).T.astype(bf)),
        "w21T": np.ascontiguousarray(np.asarray(w21, np.float32).T.astype(bf)),
        "w23T": np.ascontiguousarray(np.asarray(w23, np.float32).T.astype(bf)),
        "w3T": np.ascontiguousarray(np.asarray(w3, np.float32).T.astype(bf)),
        "b1": col(b1), "b23e": col(b23e.astype(np.float32)), "b3": col(b3),
        "n1w": col(n1w), "n1b": col(n1b), "n2w": col(n2w), "n2b": col(n2b),
    }
    xs = np.asarray(x, np.float32).astype(bf)
    in_maps = [dict(common, x=np.ascontiguousarray(xs[i].reshape(C, N)))
               for i in range(8)]
    trace = bool(os.environ.get("KPROF"))
    ncores = int(os.environ.get("NCORES", "8"))
    res = run_bass_kernel_spmd(nc, in_maps[:ncores], core_ids=list(range(ncores)),
                               trace=trace)
    if trace:
        print("HW exec time:", res.exec_time_ns, "ns")
        print("profile trace_dir:", getattr(res, "profile_json", None))
    outs = [np.asarray(res.results[i]["out"], np.float32).reshape(C, R, R, R)
            for i in range(len(res.results))]
    while len(outs) < 8:
        outs.append(outs[0])
    return np.stack(outs)
